# revision 22
# baseline (speedup 1.0000x reference)
"""Single-head causal attention (B=4, S=4096, E=768, H=64) on 8 TRN2 cores.

Sharding: 4 batches x 2 cores; each core handles its batch's interleaved
key half (local key blocks [512p,512p+128) and [512p+256,512p+384) in a
rolled-by-128h layout), computes partial numerator/denominator over its
keys for ALL queries; host combines the two partials per batch.
K/V biases are added on host (per-query-constant score shifts don't change
softmax; attn rows sum to 1 so V bias adds directly).

v3 changes vs v2 (baseline 74.3us):
- Startup: wqk+bq8 DMAs first, block-0 xt loaded as 6 per-chunk DMAs so
  the first QK matmuls start ~1.7us instead of ~7us (DMA queue is serial).
- Diagonal pair trimmed: half-1 scores matmul only computes the 256 live
  query columns, exp runs on [128,768] not [128,1024], fully-masked PV
  matmuls skipped.
- Causal masking via a precomputed [128,128] bf16 triangle multiplied on
  DVE (127ns) instead of gpsimd affine_select on [128,512] (~1.3us each,
  was on the exp->PV critical path).
- k_out/v_out DMAs issued as soon as kt2/vaug are complete (overlap the
  last attention block) instead of after everything.
- PV optionally 4x column-tiled (PV_COLTILE): 4 concurrent matmuls with
  [128,32] stationaries cut the per-matmul LDWEIGHTS cost.
"""

import numpy as np

import concourse.bass as bass
import concourse.tile as tile
from concourse import bacc, mybir, bass_utils

F32 = mybir.dt.float32
BF16 = mybir.dt.bfloat16
AF = mybir.ActivationFunctionType

B, S, E, H = 4, 4096, 768, 64
EC = E // 128        # e-chunks (6)
NSB = S // 512       # s-blocks / q-blocks (8)
NKC = S // 256       # own key chunks per core (16)

PV_COLTILE = False
PV_FP8 = True       # non-diag ptile in fp8e4m3 (exp shifted by -3; shift
                    # cancels in num/den), halves PV LDWEIGHTS cost
FP8 = mybir.dt.float8e4
EXP_BIAS = -3.0


def build_nc(reps=None):
    nc = bacc.Bacc("TRN2", target_bir_lowering=False, debug=False, num_devices=8)
    xt = nc.dram_tensor("xt", [E, S], BF16, kind="ExternalInput").ap()
    wqk = nc.dram_tensor("wqk", [E, 2 * H], BF16, kind="ExternalInput").ap()
    wv = nc.dram_tensor("wv", [E, H], BF16, kind="ExternalInput").ap()
    bq8 = nc.dram_tensor("bq8", [H, 1], F32, kind="ExternalInput").ap()
    r_out = nc.dram_tensor("r_out", [S, H + 1], F32, kind="ExternalOutput").ap()
    k_out = nc.dram_tensor("k_out", [128, S // 4], BF16, kind="ExternalOutput").ap()
    v_out = nc.dram_tensor("v_out", [NKC, 128, H], BF16, kind="ExternalOutput").ap()

    xt_r = xt.rearrange("(c p) s -> p c s", p=128)
    wqk_r = wqk.rearrange("(c p) h -> p c h", p=128)
    wv_r = wv.rearrange("(c p) h -> p c h", p=128)

    MUL, ADD = mybir.AluOpType.mult, mybir.AluOpType.add

    with tile.TileContext(nc) as tc:
        with (
            tc.tile_pool(name="consts", bufs=1) as consts,
            tc.tile_pool(name="persist", bufs=1) as persist,
        ):
            # ---- constants (wqk + bq8 first: needed by block-0 proj) ----
            wqk_sb = consts.tile([128, EC, 2 * H], BF16)
            nc.sync.dma_start(out=wqk_sb, in_=wqk_r)
            bq8_sb = consts.tile([H, 1], F32)
            nc.scalar.dma_start(out=bq8_sb, in_=bq8)

            # ---- persistent per-iteration state ----
            kt2 = persist.tile([128, S // 4], BF16)
            qt2 = persist.tile([128, S], BF16)      # Q^T duplicated rows
            vaug = persist.tile([128, NKC, H + 1], BF16)
            osb_all = persist.tile([128, NSB, 4, H + 1], F32)

            wv_sb = consts.tile([128, EC, H], BF16)
            nc.scalar.dma_start(out=wv_sb, in_=wv_r)
            ones_bf = consts.tile([128, NKC], BF16)
            nc.vector.memset(ones_bf, 1.0)
            nc.vector.tensor_copy(vaug[:, :, H], ones_bf)
            ebias = consts.tile([128, 1], F32)
            nc.vector.memset(ebias, EXP_BIAS)
            # [128,128] lower-keep triangle: tri[k,q]=1 iff k<=q
            tri = consts.tile([128, 128], BF16)
            nc.gpsimd.memset(tri, 1.0)
            nc.gpsimd.affine_select(
                out=tri, in_=tri,
                compare_op=mybir.AluOpType.is_ge,
                fill=0.0, base=0, pattern=[[1, 128]],
                channel_multiplier=-1,
            )

            def body():
                with (
                    tc.tile_pool(name="xt0_pool", bufs=2) as xt0_pool,
                    tc.tile_pool(name="xt_pool", bufs=2) as xt_pool,
                    tc.tile_pool(name="pt_pool", bufs=3) as pt_pool,
                    tc.tile_pool(name="pt8_pool", bufs=5) as pt8_pool,
                    tc.tile_pool(name="ps_mm", bufs=2, space="PSUM") as ps_mm,
                    tc.tile_pool(name="ps_qk", bufs=2, space="PSUM") as ps_qk,
                    tc.tile_pool(name="ps_v", bufs=1, space="PSUM") as ps_v,
                    tc.tile_pool(name="ps_o", bufs=1, space="PSUM") as ps_o,
                ):
                    # block-0 xt in two 3-chunk DMAs: the first QK matmuls
                    # start while the second half is still in flight
                    xt0 = [xt0_pool.tile([128, 3, 512], BF16, name=f"xt0_{h}",
                                         tag=f"xt0_{h}") for h in range(2)]
                    for h in range(2):
                        nc.sync.dma_start(out=xt0[h],
                                          in_=xt_r[:, 3 * h:3 * h + 3, :512])


                    def xt_chunk(st, G, c, lo, hi):
                        """bf16 xt slice [128, lo:hi] of block G, chunk c."""
                        if G == 0:
                            return xt0[c // 3][:, c % 3, lo:hi]
                        return st["xt"][:, c, lo:hi]

                    def proj_steps(G):
                        s0 = G * 512
                        st = {}
                        steps = []

                        def s_dma():
                            if G > 0:
                                st["xt"] = xt_pool.tile([128, EC, 512], BF16,
                                                        name="xt_t", tag="xt")
                                nc.sync.dma_start(out=st["xt"],
                                                  in_=xt_r[:, :, s0:s0 + 512])
                            st["psqk"] = ps_qk.tile([128, 512], F32,
                                                    name="psqk", tag="psqk")
                        steps.append(s_dma)
                        for c in range(EC):
                            def s_mm(c=c):
                                nc.tensor.matmul(
                                    st["psqk"], wqk_sb[:, c, :],
                                    xt_chunk(st, G, c, 0, 512),
                                    start=(c == 0), stop=(c == EC - 1),
                                )
                            steps.append(s_mm)

                        def s_dve():
                            nc.vector.tensor_scalar(
                                out=qt2[0:H, s0:s0 + 512],
                                in0=st["psqk"][0:H, :],
                                scalar1=0.125, scalar2=bq8_sb,
                                op0=MUL, op1=ADD,
                            )
                            nc.vector.tensor_copy(
                                qt2[H:2 * H, s0:s0 + 512],
                                qt2[0:H, s0:s0 + 512])
                            nc.vector.tensor_copy(
                                kt2[0:H, 128 * G:128 * G + 128],
                                st["psqk"][H:2 * H, 0:128])
                            nc.vector.tensor_copy(
                                kt2[H:2 * H, 128 * G:128 * G + 128],
                                st["psqk"][H:2 * H, 256:384])
                            if G == NSB - 1:
                                nc.scalar.dma_start(out=k_out, in_=kt2)
                        steps.append(s_dve)

                        def s_vnat(ji, j):
                            if ji == 0:
                                st["psv"] = ps_v.tile([128, 2, H], F32,
                                                      name="psv", tag="psv")
                            for c in range(EC):
                                nc.tensor.matmul(
                                    st["psv"][:, ji, :],
                                    xt_chunk(st, G, c, j * 128, (j + 1) * 128),
                                    wv_sb[:, c, :],
                                    start=(ji == 0 and c == 0),
                                    stop=(ji == 1 and c == EC - 1),
                                    skip_group_check=True,
                                )
                            if ji == 1:
                                nc.vector.tensor_copy(
                                    vaug[:, 2 * G:2 * G + 2, 0:H],
                                    st["psv"])
                                if G == NSB - 1:
                                    nc.scalar.dma_start(
                                        out=v_out.rearrange("s p h -> p s h"),
                                        in_=vaug[:, :, 0:H])
                        steps.append(lambda: s_vnat(0, 0))
                        steps.append(lambda: s_vnat(1, 2))
                        return steps

                    def emit_attn(G, pending):
                        s0 = G * 512
                        pso = ps_o.tile([128, 4, H + 1], F32, tag="pso")
                        nmm = [0]

                        def emit_scores(p):
                            pss = ps_mm.tile([128, 1024], F32, tag="mm1k")
                            nc.tensor.matmul(
                                pss[:, 0:512],
                                kt2[0:H, 128 * p:128 * p + 128],
                                qt2[0:H, s0:s0 + 512],
                                start=True, stop=True,
                            )
                            if p == G:   # diag: only queries 256: for half 1
                                nc.tensor.matmul(
                                    pss[:, 512:768],
                                    kt2[H:2 * H, 128 * p:128 * p + 128],
                                    qt2[H:2 * H, s0 + 256:s0 + 512],
                                    start=True, stop=True,
                                )
                            else:
                                nc.tensor.matmul(
                                    pss[:, 512:1024],
                                    kt2[H:2 * H, 128 * p:128 * p + 128],
                                    qt2[H:2 * H, s0:s0 + 512],
                                    start=True, stop=True,
                                )
                            return pss

                        def pv_groups(p):
                            # (ptile col base, vaug slot, q sub-block j, mask?)
                            if p == G:
                                return ([(j * 128, 2 * p, j, j == 0)
                                         for j in range(4)] +
                                        [(512, 2 * p + 1, 2, True),
                                         (640, 2 * p + 1, 3, False)])
                            return ([(j * 128, 2 * p, j, False)
                                     for j in range(4)] +
                                    [(512 + j * 128, 2 * p + 1, j, False)
                                     for j in range(4)])

                        def emit_rest(p, pss, n_total):
                            fd = 768 if p == G else 1024
                            if PV_FP8 and p != G:
                                ptile = pt8_pool.tile([128, 1024], FP8,
                                                      tag="pt8")
                            else:
                                ptile = pt_pool.tile([128, 1024], BF16,
                                                     tag="pt")
                            nc.scalar.activation(ptile[:, 0:fd], pss[:, 0:fd],
                                                 AF.Exp, bias=ebias,
                                                 scale=1.0)
                            groups = pv_groups(p)
                            for base, slot, j, need_mask in groups:
                                if need_mask:
                                    nc.vector.tensor_mul(
                                        ptile[:, base:base + 128],
                                        ptile[:, base:base + 128], tri)
                            for base, slot, j, _ in groups:
                                first = nmm[0] == 0
                                nmm[0] += 1
                                last = nmm[0] == n_total
                                if PV_COLTILE:
                                    for t in range(4):
                                        nc.tensor.matmul(
                                            pso[32 * t:32 * t + 32, j, :],
                                            ptile[:, base + 32 * t:
                                                  base + 32 * t + 32],
                                            vaug[:, slot, :],
                                            start=(first and t == 0),
                                            stop=(last and t == 3),
                                            skip_group_check=True,
                                            tile_position=(0, 32 * t),
                                        )
                                else:
                                    nc.tensor.matmul(
                                        pso[:, j, :],
                                        ptile[:, base:base + 128],
                                        vaug[:, slot, :],
                                        start=first, stop=last,
                                        skip_group_check=True,
                                    )

                        order = [0, G] + list(range(1, G)) if G >= 1 else [0]
                        n_total = 8 * len(order) - 2   # diag pair drops 2
                        prev = emit_scores(order[0])
                        for oi in range(1, len(order)):
                            cur = emit_scores(order[oi])
                            emit_rest(order[oi - 1], prev, n_total)
                            if pending:
                                pending.pop(0)()
                            prev = cur
                        emit_rest(order[-1], prev, n_total)
                        if pending:
                            pending.pop(0)()
                        nc.vector.tensor_copy(osb_all[:, G], pso)

                    for st in proj_steps(0):
                        st()
                    for G in range(NSB):
                        pending = proj_steps(G + 1) if G + 1 < NSB else []
                        emit_attn(G, pending)
                        for st in pending:
                            st()
                    nc.scalar.dma_start(
                        out=r_out.rearrange("(g j p) c -> p g j c",
                                            p=128, j=4),
                        in_=osb_all)

            if reps is None:
                body()
            else:
                unroll = 4 if reps % 4 == 0 else (2 if reps % 2 == 0 else 1)
                with tc.For_i(0, reps // unroll, 1,
                              hint_engines=(mybir.EngineType.PE,),
                              staggered_reset=True):
                    for _ in range(unroll):
                        body()

    nc.compile()
    return nc


def _bf16(a):
    import ml_dtypes
    return np.asarray(a, dtype=ml_dtypes.bfloat16)


def _prep_inputs(x, wq_w, wq_b, wk_w, wk_b, wv_w, wv_b):
    x = np.asarray(x, np.float32)
    wqk = np.ascontiguousarray(
        np.concatenate([np.asarray(wq_w), np.asarray(wk_w)], axis=1)
    ).astype(np.float32)
    bq8 = np.ascontiguousarray(
        np.asarray(wq_b, np.float32) / 8.0).reshape(H, 1)
    wqk_b = _bf16(wqk)
    wv_b16 = _bf16(np.asarray(wv_w, np.float32))
    in_maps = []
    for c in range(8):
        b, h = c // 2, c % 2
        xr = np.roll(x[b], -128 * h, axis=0)            # [S, E]
        xt = _bf16(np.ascontiguousarray(xr.T))          # [E, S]
        in_maps.append({
            "xt": xt, "wqk": wqk_b, "wv": wv_b16, "bq8": bq8,
        })
    return in_maps


def kernel(x, wq_w, wq_b, wk_w, wk_b, wv_w, wv_b):
    nc = build_nc()
    in_maps = _prep_inputs(x, wq_w, wq_b, wk_w, wk_b, wv_w, wv_b)
    res = bass_utils.run_bass_kernel_spmd(nc, in_maps, core_ids=list(range(8)))

    bk = np.asarray(wk_b, np.float32)
    bv = np.asarray(wv_b, np.float32)
    result = np.empty((B, S, H), np.float32)
    K = np.empty((B, S, H), np.float32)
    V = np.empty((B, S, H), np.float32)
    for b in range(B):
        num = np.zeros((S, H), np.float64)
        den = np.zeros((S,), np.float64)
        for h in range(2):
            r = np.asarray(res.results[2 * b + h]["r_out"], np.float64)
            r = np.roll(r, 128 * h, axis=0)     # local -> global queries
            if h == 1:
                r[0:128, :] = 0.0               # wrapped queries: garbage
            num += r[:, 0:H]
            den += r[:, H]
            ko = np.asarray(res.results[2 * b + h]["k_out"], np.float32)
            vo = np.asarray(res.results[2 * b + h]["v_out"], np.float32)
            i = np.arange(128)
            for p in range(8):
                K[b, 512 * p + 128 * h + i] = ko[0:H, 128 * p + i].T
                K[b, 512 * p + 256 + 128 * h + i] = ko[H:2 * H, 128 * p + i].T
            idx = (np.arange(NKC) * 256 + 128 * h)[:, None] + i
            V[b, idx.ravel()] = vo.reshape(S // 2, H)
        result[b] = (num / den[:, None]).astype(np.float32)
    K += bk
    V += bv
    result += bv
    return result, K, V


# revision 23
# speedup vs baseline: 1.4617x; 1.4617x over previous
"""Single-head causal attention (B=4, S=4096, E=768, H=64) on 8 TRN2 cores.

Sharding: 4 batches x 2 cores; each core handles its batch's interleaved
key half (local key blocks [512p,512p+128) and [512p+256,512p+384) in a
rolled-by-128h layout), computes partial numerator/denominator over its
keys for ALL queries; host combines the two partials per batch.
K/V biases are added on host (per-query-constant score shifts don't change
softmax; attn rows sum to 1 so V bias adds directly).

v3 changes vs v2 (baseline 74.3us):
- Startup: wqk+bq8 DMAs first, block-0 xt loaded as 6 per-chunk DMAs so
  the first QK matmuls start ~1.7us instead of ~7us (DMA queue is serial).
- Diagonal pair trimmed: half-1 scores matmul only computes the 256 live
  query columns, exp runs on [128,768] not [128,1024], fully-masked PV
  matmuls skipped.
- Causal masking via a precomputed [128,128] bf16 triangle multiplied on
  DVE (127ns) instead of gpsimd affine_select on [128,512] (~1.3us each,
  was on the exp->PV critical path).
- k_out/v_out DMAs issued as soon as kt2/vaug are complete (overlap the
  last attention block) instead of after everything.
- PV optionally 4x column-tiled (PV_COLTILE): 4 concurrent matmuls with
  [128,32] stationaries cut the per-matmul LDWEIGHTS cost.
"""

import numpy as np

import concourse.bass as bass
import concourse.tile as tile
from concourse import bacc, mybir, bass_utils

F32 = mybir.dt.float32
BF16 = mybir.dt.bfloat16
AF = mybir.ActivationFunctionType

B, S, E, H = 4, 4096, 768, 64
EC = E // 128        # e-chunks (6)
NSB = S // 512       # s-blocks / q-blocks (8)
NKC = S // 256       # own key chunks per core (16)

PV_COLTILE = False
PV_FP8 = True       # non-diag ptile in fp8e4m3 (exp shifted by -3; shift
                    # cancels in num/den), halves PV LDWEIGHTS cost
FP8 = mybir.dt.float8e4
EXP_BIAS = -3.0


def build_nc(reps=None):
    nc = bacc.Bacc("TRN2", target_bir_lowering=False, debug=False, num_devices=8)
    xt = nc.dram_tensor("xt", [E, S], BF16, kind="ExternalInput").ap()
    wqk = nc.dram_tensor("wqk", [E, 2 * H], BF16, kind="ExternalInput").ap()
    wv = nc.dram_tensor("wv", [E, H], BF16, kind="ExternalInput").ap()
    bq8 = nc.dram_tensor("bq8", [H, 1], F32, kind="ExternalInput").ap()
    r_out = nc.dram_tensor("r_out", [128, NSB, 4, H + 1], F32,
                       kind="ExternalOutput").ap()
    k_out = nc.dram_tensor("k_out", [128, S // 4], BF16, kind="ExternalOutput").ap()
    v_out = nc.dram_tensor("v_out", [NKC, 128, H], BF16, kind="ExternalOutput").ap()

    xt_r = xt.rearrange("(c p) s -> p c s", p=128)
    wqk_r = wqk.rearrange("(c p) h -> p c h", p=128)
    wv_r = wv.rearrange("(c p) h -> p c h", p=128)

    MUL, ADD = mybir.AluOpType.mult, mybir.AluOpType.add

    with tile.TileContext(nc) as tc:
        with (
            tc.tile_pool(name="consts", bufs=1) as consts,
            tc.tile_pool(name="persist", bufs=1) as persist,
        ):
            # ---- constants (wqk + bq8 first: needed by block-0 proj) ----
            wqk_sb = consts.tile([128, EC, 2 * H], BF16)
            nc.sync.dma_start(out=wqk_sb, in_=wqk_r)
            bq8_sb = consts.tile([H, 1], F32)
            nc.scalar.dma_start(out=bq8_sb, in_=bq8)

            # ---- persistent per-iteration state ----
            kt2 = persist.tile([128, S // 4], BF16)
            qt2 = persist.tile([128, S], BF16)      # Q^T duplicated rows
            vaug = persist.tile([128, NKC, H + 1], BF16)

            wv_sb = consts.tile([128, EC, H], BF16)
            nc.scalar.dma_start(out=wv_sb, in_=wv_r)
            ones_bf = consts.tile([128, NKC], BF16)
            nc.vector.memset(ones_bf, 1.0)
            nc.vector.tensor_copy(vaug[:, :, H], ones_bf)
            ebias = consts.tile([128, 1], F32)
            nc.vector.memset(ebias, EXP_BIAS)
            # [128,128] lower-keep triangle: tri[k,q]=1 iff k<=q
            tri = consts.tile([128, 128], BF16)
            nc.gpsimd.memset(tri, 1.0)
            nc.gpsimd.affine_select(
                out=tri, in_=tri,
                compare_op=mybir.AluOpType.is_ge,
                fill=0.0, base=0, pattern=[[1, 128]],
                channel_multiplier=-1,
            )

            def body():
                with (
                    tc.tile_pool(name="xt0_pool", bufs=2) as xt0_pool,
                    tc.tile_pool(name="xt_pool", bufs=2) as xt_pool,
                    tc.tile_pool(name="pt_pool", bufs=3) as pt_pool,
                    tc.tile_pool(name="ob_pool", bufs=2) as ob_pool,
                    tc.tile_pool(name="pt8_pool", bufs=5) as pt8_pool,
                    tc.tile_pool(name="ps_mm", bufs=2, space="PSUM") as ps_mm,
                    tc.tile_pool(name="ps_qk", bufs=2, space="PSUM") as ps_qk,
                    tc.tile_pool(name="ps_v", bufs=1, space="PSUM") as ps_v,
                    tc.tile_pool(name="ps_o", bufs=1, space="PSUM") as ps_o,
                ):
                    # block-0 xt in two 3-chunk DMAs: the first QK matmuls
                    # start while the second half is still in flight
                    xt0 = [xt0_pool.tile([128, 3, 512], BF16, name=f"xt0_{h}",
                                         tag=f"xt0_{h}") for h in range(2)]
                    for h in range(2):
                        nc.sync.dma_start(out=xt0[h],
                                          in_=xt_r[:, 3 * h:3 * h + 3, :512])


                    def xt_chunk(st, G, c, lo, hi):
                        """bf16 xt slice [128, lo:hi] of block G, chunk c."""
                        if G == 0:
                            return xt0[c // 3][:, c % 3, lo:hi]
                        return st["xt"][:, c, lo:hi]

                    def proj_steps(G):
                        s0 = G * 512
                        st = {}
                        steps = []

                        def s_dma():
                            if G > 0:
                                st["xt"] = xt_pool.tile([128, EC, 512], BF16,
                                                        name="xt_t", tag="xt")
                                nc.sync.dma_start(out=st["xt"],
                                                  in_=xt_r[:, :, s0:s0 + 512])
                            st["psqk"] = ps_qk.tile([128, 512], F32,
                                                    name="psqk", tag="psqk")
                        steps.append(s_dma)
                        for c in range(EC):
                            def s_mm(c=c):
                                nc.tensor.matmul(
                                    st["psqk"], wqk_sb[:, c, :],
                                    xt_chunk(st, G, c, 0, 512),
                                    start=(c == 0), stop=(c == EC - 1),
                                )
                            steps.append(s_mm)

                        def s_dve():
                            nc.vector.tensor_scalar(
                                out=qt2[0:H, s0:s0 + 512],
                                in0=st["psqk"][0:H, :],
                                scalar1=0.125, scalar2=bq8_sb,
                                op0=MUL, op1=ADD,
                            )
                            nc.vector.tensor_copy(
                                qt2[H:2 * H, s0:s0 + 512],
                                qt2[0:H, s0:s0 + 512])
                            nc.vector.tensor_copy(
                                kt2[0:H, 128 * G:128 * G + 128],
                                st["psqk"][H:2 * H, 0:128])
                            nc.vector.tensor_copy(
                                kt2[H:2 * H, 128 * G:128 * G + 128],
                                st["psqk"][H:2 * H, 256:384])
                            if G == NSB - 1:
                                nc.gpsimd.dma_start(out=k_out, in_=kt2)
                        steps.append(s_dve)

                        def s_vnat(ji, j):
                            if ji == 0:
                                st["psv"] = ps_v.tile([128, 2, H], F32,
                                                      name="psv", tag="psv")
                            for c in range(EC):
                                nc.tensor.matmul(
                                    st["psv"][:, ji, :],
                                    xt_chunk(st, G, c, j * 128, (j + 1) * 128),
                                    wv_sb[:, c, :],
                                    start=(ji == 0 and c == 0),
                                    stop=(ji == 1 and c == EC - 1),
                                    skip_group_check=True,
                                )
                            if ji == 1:
                                nc.vector.tensor_copy(
                                    vaug[:, 2 * G:2 * G + 2, 0:H],
                                    st["psv"])
                                if G == NSB - 1:
                                    nc.gpsimd.dma_start(
                                        out=v_out.rearrange("s p h -> p s h"),
                                        in_=vaug[:, :, 0:H])
                        steps.append(lambda: s_vnat(0, 0))
                        steps.append(lambda: s_vnat(1, 2))
                        return steps

                    def emit_attn(G, pending):
                        s0 = G * 512
                        pso = ps_o.tile([128, 4, H + 1], F32, tag="pso")
                        nmm = [0]

                        def emit_scores(p):
                            pss = ps_mm.tile([128, 1024], F32, tag="mm1k")
                            nc.tensor.matmul(
                                pss[:, 0:512],
                                kt2[0:H, 128 * p:128 * p + 128],
                                qt2[0:H, s0:s0 + 512],
                                start=True, stop=True,
                            )
                            if p == G:   # diag: only queries 256: for half 1
                                nc.tensor.matmul(
                                    pss[:, 512:768],
                                    kt2[H:2 * H, 128 * p:128 * p + 128],
                                    qt2[H:2 * H, s0 + 256:s0 + 512],
                                    start=True, stop=True,
                                )
                            else:
                                nc.tensor.matmul(
                                    pss[:, 512:1024],
                                    kt2[H:2 * H, 128 * p:128 * p + 128],
                                    qt2[H:2 * H, s0:s0 + 512],
                                    start=True, stop=True,
                                )
                            return pss

                        def pv_groups(p):
                            # (ptile col base, vaug slot, q sub-block j, mask?)
                            if p == G:
                                return ([(j * 128, 2 * p, j, j == 0)
                                         for j in range(4)] +
                                        [(512, 2 * p + 1, 2, True),
                                         (640, 2 * p + 1, 3, False)])
                            return ([(j * 128, 2 * p, j, False)
                                     for j in range(4)] +
                                    [(512 + j * 128, 2 * p + 1, j, False)
                                     for j in range(4)])

                        def emit_rest(p, pss, n_total):
                            fd = 768 if p == G else 1024
                            if PV_FP8 and p != G:
                                ptile = pt8_pool.tile([128, 1024], FP8,
                                                      tag="pt8")
                            else:
                                ptile = pt_pool.tile([128, 1024], BF16,
                                                     tag="pt")
                            nc.scalar.activation(ptile[:, 0:fd], pss[:, 0:fd],
                                                 AF.Exp, bias=ebias,
                                                 scale=1.0)
                            groups = pv_groups(p)
                            for base, slot, j, need_mask in groups:
                                if need_mask:
                                    nc.vector.tensor_mul(
                                        ptile[:, base:base + 128],
                                        ptile[:, base:base + 128], tri)
                            for base, slot, j, _ in groups:
                                first = nmm[0] == 0
                                nmm[0] += 1
                                last = nmm[0] == n_total
                                if PV_COLTILE:
                                    for t in range(4):
                                        nc.tensor.matmul(
                                            pso[32 * t:32 * t + 32, j, :],
                                            ptile[:, base + 32 * t:
                                                  base + 32 * t + 32],
                                            vaug[:, slot, :],
                                            start=(first and t == 0),
                                            stop=(last and t == 3),
                                            skip_group_check=True,
                                            tile_position=(0, 32 * t),
                                        )
                                else:
                                    nc.tensor.matmul(
                                        pso[:, j, :],
                                        ptile[:, base:base + 128],
                                        vaug[:, slot, :],
                                        start=first, stop=last,
                                        skip_group_check=True,
                                    )

                        order = [0, G] + list(range(1, G)) if G >= 1 else [0]
                        n_total = 8 * len(order) - 2   # diag pair drops 2
                        prev = emit_scores(order[0])
                        for oi in range(1, len(order)):
                            cur = emit_scores(order[oi])
                            emit_rest(order[oi - 1], prev, n_total)
                            if pending:
                                pending.pop(0)()
                            prev = cur
                        emit_rest(order[-1], prev, n_total)
                        if pending:
                            pending.pop(0)()
                        osb = ob_pool.tile([128, 4, H + 1], F32, tag="osb")
                        nc.vector.tensor_copy(osb, pso)
                        nc.gpsimd.dma_start(out=r_out[:, G], in_=osb)

                    for st in proj_steps(0):
                        st()
                    for G in range(NSB):
                        pending = proj_steps(G + 1) if G + 1 < NSB else []
                        emit_attn(G, pending)
                        for st in pending:
                            st()

            if reps is None:
                body()
            else:
                unroll = 4 if reps % 4 == 0 else (2 if reps % 2 == 0 else 1)
                with tc.For_i(0, reps // unroll, 1,
                              hint_engines=(mybir.EngineType.PE,),
                              staggered_reset=True):
                    for _ in range(unroll):
                        body()

    nc.compile()
    return nc


def _bf16(a):
    import ml_dtypes
    return np.asarray(a, dtype=ml_dtypes.bfloat16)


def _prep_inputs(x, wq_w, wq_b, wk_w, wk_b, wv_w, wv_b):
    x = np.asarray(x, np.float32)
    wqk = np.ascontiguousarray(
        np.concatenate([np.asarray(wq_w), np.asarray(wk_w)], axis=1)
    ).astype(np.float32)
    bq8 = np.ascontiguousarray(
        np.asarray(wq_b, np.float32) / 8.0).reshape(H, 1)
    wqk_b = _bf16(wqk)
    wv_b16 = _bf16(np.asarray(wv_w, np.float32))
    in_maps = []
    for c in range(8):
        b, h = c // 2, c % 2
        xr = np.roll(x[b], -128 * h, axis=0)            # [S, E]
        xt = _bf16(np.ascontiguousarray(xr.T))          # [E, S]
        in_maps.append({
            "xt": xt, "wqk": wqk_b, "wv": wv_b16, "bq8": bq8,
        })
    return in_maps


def kernel(x, wq_w, wq_b, wk_w, wk_b, wv_w, wv_b):
    nc = build_nc()
    in_maps = _prep_inputs(x, wq_w, wq_b, wk_w, wk_b, wv_w, wv_b)
    res = bass_utils.run_bass_kernel_spmd(nc, in_maps, core_ids=list(range(8)))

    bk = np.asarray(wk_b, np.float32)
    bv = np.asarray(wv_b, np.float32)
    result = np.empty((B, S, H), np.float32)
    K = np.empty((B, S, H), np.float32)
    V = np.empty((B, S, H), np.float32)
    for b in range(B):
        num = np.zeros((S, H), np.float64)
        den = np.zeros((S,), np.float64)
        for h in range(2):
            r = np.asarray(res.results[2 * b + h]["r_out"], np.float64)
            r = r.transpose(1, 2, 0, 3).reshape(S, H + 1)
            r = np.roll(r, 128 * h, axis=0)     # local -> global queries
            if h == 1:
                r[0:128, :] = 0.0               # wrapped queries: garbage
            num += r[:, 0:H]
            den += r[:, H]
            ko = np.asarray(res.results[2 * b + h]["k_out"], np.float32)
            vo = np.asarray(res.results[2 * b + h]["v_out"], np.float32)
            i = np.arange(128)
            for p in range(8):
                K[b, 512 * p + 128 * h + i] = ko[0:H, 128 * p + i].T
                K[b, 512 * p + 256 + 128 * h + i] = ko[H:2 * H, 128 * p + i].T
            idx = (np.arange(NKC) * 256 + 128 * h)[:, None] + i
            V[b, idx.ravel()] = vo.reshape(S // 2, H)
        result[b] = (num / den[:, None]).astype(np.float32)
    K += bk
    V += bv
    result += bv
    return result, K, V


# revision 24
# speedup vs baseline: 1.5702x; 1.0742x over previous
"""Single-head causal attention (B=4, S=4096, E=768, H=64) on 8 TRN2 cores.

Sharding: 4 batches x 2 cores; each core handles its batch's interleaved
key half (local key blocks [512p,512p+128) and [512p+256,512p+384) in a
rolled-by-128h layout), computes partial numerator/denominator over its
keys for ALL queries; host combines the two partials per batch.
K/V biases are added on host (per-query-constant score shifts don't change
softmax; attn rows sum to 1 so V bias adds directly).

v3 changes vs v2 (baseline 74.3us):
- Startup: wqk+bq8 DMAs first, block-0 xt loaded as 6 per-chunk DMAs so
  the first QK matmuls start ~1.7us instead of ~7us (DMA queue is serial).
- Diagonal pair trimmed: half-1 scores matmul only computes the 256 live
  query columns, exp runs on [128,768] not [128,1024], fully-masked PV
  matmuls skipped.
- Causal masking via a precomputed [128,128] bf16 triangle multiplied on
  DVE (127ns) instead of gpsimd affine_select on [128,512] (~1.3us each,
  was on the exp->PV critical path).
- k_out/v_out DMAs issued as soon as kt2/vaug are complete (overlap the
  last attention block) instead of after everything.
- PV optionally 4x column-tiled (PV_COLTILE): 4 concurrent matmuls with
  [128,32] stationaries cut the per-matmul LDWEIGHTS cost.
"""

import numpy as np

import concourse.bass as bass
import concourse.tile as tile
from concourse import bacc, mybir, bass_utils

F32 = mybir.dt.float32
BF16 = mybir.dt.bfloat16
AF = mybir.ActivationFunctionType

B, S, E, H = 4, 4096, 768, 64
EC = E // 128        # e-chunks (6)
NSB = S // 512       # s-blocks / q-blocks (8)
NKC = S // 256       # own key chunks per core (16)

PV_COLTILE = False
PV_FP8 = True       # non-diag ptile in fp8e4m3 (exp shifted by -3; shift
                    # cancels in num/den), halves PV LDWEIGHTS cost
FP8 = mybir.dt.float8e4
EXP_BIAS = -3.0


def build_nc(reps=None):
    nc = bacc.Bacc("TRN2", target_bir_lowering=False, debug=False, num_devices=8)
    xt = nc.dram_tensor("xt", [E, S], BF16, kind="ExternalInput").ap()
    wqk = nc.dram_tensor("wqk", [E, 2 * H], BF16, kind="ExternalInput").ap()
    wv = nc.dram_tensor("wv", [E, H], BF16, kind="ExternalInput").ap()
    bq8 = nc.dram_tensor("bq8", [H, 1], F32, kind="ExternalInput").ap()
    r_out = nc.dram_tensor("r_out", [128, NSB, 4, H + 1], F32,
                       kind="ExternalOutput").ap()
    k_out = nc.dram_tensor("k_out", [128, S // 4], BF16, kind="ExternalOutput").ap()
    v_out = nc.dram_tensor("v_out", [NKC, 128, H], BF16, kind="ExternalOutput").ap()

    xt_r = xt.rearrange("(c p) s -> p c s", p=128)
    wqk_r = wqk.rearrange("(c p) h -> p c h", p=128)
    wv_r = wv.rearrange("(c p) h -> p c h", p=128)

    MUL, ADD = mybir.AluOpType.mult, mybir.AluOpType.add

    with tile.TileContext(nc) as tc:
        with (
            tc.tile_pool(name="consts", bufs=1) as consts,
            tc.tile_pool(name="persist", bufs=1) as persist,
        ):
            # ---- constants (wqk + bq8 first: needed by block-0 proj) ----
            wqk_sb = consts.tile([128, EC, 2 * H], BF16)
            nc.sync.dma_start(out=wqk_sb, in_=wqk_r)
            bq8_sb = consts.tile([H, 1], F32)
            nc.scalar.dma_start(out=bq8_sb, in_=bq8)

            # ---- persistent per-iteration state ----
            kt2 = persist.tile([128, S // 4], BF16)
            qt2 = persist.tile([128, S], BF16)      # Q^T duplicated rows
            vaug = persist.tile([128, NKC, H + 1], BF16)

            wv_sb = consts.tile([128, EC, H], BF16)
            nc.scalar.dma_start(out=wv_sb, in_=wv_r)
            ones_bf = consts.tile([128, NKC], BF16)
            nc.vector.memset(ones_bf, 1.0)
            nc.vector.tensor_copy(vaug[:, :, H], ones_bf)
            ebias = consts.tile([128, 1], F32)
            nc.vector.memset(ebias, EXP_BIAS)
            # [128,128] lower-keep triangle: tri[k,q]=1 iff k<=q
            tri = consts.tile([128, 128], BF16)
            nc.gpsimd.memset(tri, 1.0)
            nc.gpsimd.affine_select(
                out=tri, in_=tri,
                compare_op=mybir.AluOpType.is_ge,
                fill=0.0, base=0, pattern=[[1, 128]],
                channel_multiplier=-1,
            )

            def body():
                with (
                    tc.tile_pool(name="xt0_pool", bufs=2) as xt0_pool,
                    tc.tile_pool(name="xt_pool", bufs=2) as xt_pool,
                    tc.tile_pool(name="pt_pool", bufs=3) as pt_pool,
                    tc.tile_pool(name="ob_pool", bufs=2) as ob_pool,
                    tc.tile_pool(name="pt8_pool", bufs=5) as pt8_pool,
                    tc.tile_pool(name="ps_mm", bufs=2, space="PSUM") as ps_mm,
                    tc.tile_pool(name="ps_qk", bufs=2, space="PSUM") as ps_qk,
                    tc.tile_pool(name="ps_v", bufs=1, space="PSUM") as ps_v,
                    tc.tile_pool(name="ps_o", bufs=1, space="PSUM") as ps_o,
                ):
                    # block-0 xt in two 3-chunk DMAs: the first QK matmuls
                    # start while the second half is still in flight
                    xt0 = [xt0_pool.tile([128, 3, 512], BF16, name=f"xt0_{h}",
                                         tag=f"xt0_{h}") for h in range(2)]
                    for h in range(2):
                        nc.sync.dma_start(out=xt0[h],
                                          in_=xt_r[:, 3 * h:3 * h + 3, :512])


                    def xt_chunk(st, G, c, lo, hi):
                        """bf16 xt slice [128, lo:hi] of block G, chunk c."""
                        if G == 0:
                            return xt0[c // 3][:, c % 3, lo:hi]
                        return st["xt"][:, c, lo:hi]

                    def proj_steps(G):
                        s0 = G * 512
                        st = {}
                        steps = []

                        def s_dma():
                            if G > 0:
                                st["xt"] = xt_pool.tile([128, EC, 512], BF16,
                                                        name="xt_t", tag="xt")
                                nc.sync.dma_start(out=st["xt"],
                                                  in_=xt_r[:, :, s0:s0 + 512])
                            st["psqk"] = ps_qk.tile([128, 512], F32,
                                                    name="psqk", tag="psqk")
                        steps.append(s_dma)
                        for c in range(EC):
                            def s_mm(c=c):
                                nc.tensor.matmul(
                                    st["psqk"], wqk_sb[:, c, :],
                                    xt_chunk(st, G, c, 0, 512),
                                    start=(c == 0), stop=(c == EC - 1),
                                )
                            steps.append(s_mm)

                        def s_dve():
                            nc.vector.tensor_scalar(
                                out=qt2[0:H, s0:s0 + 512],
                                in0=st["psqk"][0:H, :],
                                scalar1=0.125, scalar2=bq8_sb,
                                op0=MUL, op1=ADD,
                            )
                            nc.vector.tensor_copy(
                                qt2[H:2 * H, s0:s0 + 512],
                                qt2[0:H, s0:s0 + 512])
                            nc.vector.tensor_copy(
                                kt2[0:H, 128 * G:128 * G + 128],
                                st["psqk"][H:2 * H, 0:128])
                            nc.vector.tensor_copy(
                                kt2[H:2 * H, 128 * G:128 * G + 128],
                                st["psqk"][H:2 * H, 256:384])
                            if G == NSB - 1:
                                nc.gpsimd.dma_start(out=k_out, in_=kt2)
                        steps.append(s_dve)

                        def s_vnat(ji, j):
                            if ji == 0:
                                st["psv"] = ps_v.tile([128, 2, H], F32,
                                                      name="psv", tag="psv")
                            for c in range(EC):
                                nc.tensor.matmul(
                                    st["psv"][:, ji, :],
                                    xt_chunk(st, G, c, j * 128, (j + 1) * 128),
                                    wv_sb[:, c, :],
                                    start=(ji == 0 and c == 0),
                                    stop=(ji == 1 and c == EC - 1),
                                    skip_group_check=True,
                                )
                            if ji == 1:
                                nc.vector.tensor_copy(
                                    vaug[:, 2 * G:2 * G + 2, 0:H],
                                    st["psv"])
                                if G == NSB - 1:
                                    nc.gpsimd.dma_start(
                                        out=v_out.rearrange("s p h -> p s h"),
                                        in_=vaug[:, :, 0:H])
                        steps.append(lambda: s_vnat(0, 0))
                        steps.append(lambda: s_vnat(1, 2))
                        return steps

                    def emit_attn(G, pending):
                        s0 = G * 512
                        pso = ps_o.tile([128, 4, H + 1], F32, tag="pso")
                        nmm = [0]

                        def emit_scores(p):
                            pss = ps_mm.tile([128, 1024], F32, tag="mm1k")
                            nc.tensor.matmul(
                                pss[:, 0:512],
                                kt2[0:H, 128 * p:128 * p + 128],
                                qt2[0:H, s0:s0 + 512],
                                start=True, stop=True,
                            )
                            if p == G:   # diag: only queries 256: for half 1
                                nc.tensor.matmul(
                                    pss[:, 512:768],
                                    kt2[H:2 * H, 128 * p:128 * p + 128],
                                    qt2[H:2 * H, s0 + 256:s0 + 512],
                                    start=True, stop=True,
                                )
                            else:
                                nc.tensor.matmul(
                                    pss[:, 512:1024],
                                    kt2[H:2 * H, 128 * p:128 * p + 128],
                                    qt2[H:2 * H, s0:s0 + 512],
                                    start=True, stop=True,
                                )
                            return pss

                        def pv_groups(p):
                            # (ptile col base, vaug slot, q sub-block j, mask?)
                            if p == G:
                                return ([(j * 128, 2 * p, j, j == 0)
                                         for j in range(4)] +
                                        [(512, 2 * p + 1, 2, True),
                                         (640, 2 * p + 1, 3, False)])
                            return ([(j * 128, 2 * p, j, False)
                                     for j in range(4)] +
                                    [(512 + j * 128, 2 * p + 1, j, False)
                                     for j in range(4)])

                        def emit_rest(p, pss, n_total):
                            fd = 768 if p == G else 1024
                            if PV_FP8 and p != G:
                                ptile = pt8_pool.tile([128, 1024], FP8,
                                                      tag="pt8")
                            else:
                                ptile = pt_pool.tile([128, 1024], BF16,
                                                     tag="pt")
                            nc.scalar.activation(ptile[:, 0:fd], pss[:, 0:fd],
                                                 AF.Exp, bias=ebias,
                                                 scale=1.0)
                            groups = pv_groups(p)
                            for base, slot, j, need_mask in groups:
                                if need_mask:
                                    nc.vector.tensor_mul(
                                        ptile[:, base:base + 128],
                                        ptile[:, base:base + 128], tri)
                            for base, slot, j, _ in groups:
                                first = nmm[0] == 0
                                nmm[0] += 1
                                last = nmm[0] == n_total
                                if PV_COLTILE:
                                    for t in range(4):
                                        nc.tensor.matmul(
                                            pso[32 * t:32 * t + 32, j, :],
                                            ptile[:, base + 32 * t:
                                                  base + 32 * t + 32],
                                            vaug[:, slot, :],
                                            start=(first and t == 0),
                                            stop=(last and t == 3),
                                            skip_group_check=True,
                                            tile_position=(0, 32 * t),
                                        )
                                else:
                                    nc.tensor.matmul(
                                        pso[:, j, :],
                                        ptile[:, base:base + 128],
                                        vaug[:, slot, :],
                                        start=first, stop=last,
                                        skip_group_check=True,
                                    )

                        order = [0, G] + list(range(1, G)) if G >= 1 else [0]
                        n_total = 8 * len(order) - 2   # diag pair drops 2
                        prev = emit_scores(order[0])
                        for oi in range(1, len(order)):
                            cur = emit_scores(order[oi])
                            emit_rest(order[oi - 1], prev, n_total)
                            if pending:
                                pending.pop(0)()
                            prev = cur
                        emit_rest(order[-1], prev, n_total)
                        if pending:
                            pending.pop(0)()
                        osb = ob_pool.tile([128, 4, H + 1], F32, tag="osb")
                        nc.vector.tensor_copy(osb, pso)
                        nc.gpsimd.dma_start(out=r_out[:, G], in_=osb)

                    for st in proj_steps(0):
                        st()
                    for G in range(NSB):
                        pending = proj_steps(G + 1) if G + 1 < NSB else []
                        emit_attn(G, pending)
                        for st in pending:
                            st()

            if reps is None:
                body()
            else:
                unroll = 8 if reps % 8 == 0 else (2 if reps % 2 == 0 else 1)
                with tc.For_i(0, reps // unroll, 1,
                              hint_engines=(mybir.EngineType.PE,),
                              staggered_reset=True):
                    for _ in range(unroll):
                        body()

    nc.compile()
    return nc


def _bf16(a):
    import ml_dtypes
    return np.asarray(a, dtype=ml_dtypes.bfloat16)


def _prep_inputs(x, wq_w, wq_b, wk_w, wk_b, wv_w, wv_b):
    x = np.asarray(x, np.float32)
    wqk = np.ascontiguousarray(
        np.concatenate([np.asarray(wq_w), np.asarray(wk_w)], axis=1)
    ).astype(np.float32)
    bq8 = np.ascontiguousarray(
        np.asarray(wq_b, np.float32) / 8.0).reshape(H, 1)
    wqk_b = _bf16(wqk)
    wv_b16 = _bf16(np.asarray(wv_w, np.float32))
    in_maps = []
    for c in range(8):
        b, h = c // 2, c % 2
        xr = np.roll(x[b], -128 * h, axis=0)            # [S, E]
        xt = _bf16(np.ascontiguousarray(xr.T))          # [E, S]
        in_maps.append({
            "xt": xt, "wqk": wqk_b, "wv": wv_b16, "bq8": bq8,
        })
    return in_maps


def kernel(x, wq_w, wq_b, wk_w, wk_b, wv_w, wv_b):
    nc = build_nc()
    in_maps = _prep_inputs(x, wq_w, wq_b, wk_w, wk_b, wv_w, wv_b)
    res = bass_utils.run_bass_kernel_spmd(nc, in_maps, core_ids=list(range(8)))

    bk = np.asarray(wk_b, np.float32)
    bv = np.asarray(wv_b, np.float32)
    result = np.empty((B, S, H), np.float32)
    K = np.empty((B, S, H), np.float32)
    V = np.empty((B, S, H), np.float32)
    for b in range(B):
        num = np.zeros((S, H), np.float64)
        den = np.zeros((S,), np.float64)
        for h in range(2):
            r = np.asarray(res.results[2 * b + h]["r_out"], np.float64)
            r = r.transpose(1, 2, 0, 3).reshape(S, H + 1)
            r = np.roll(r, 128 * h, axis=0)     # local -> global queries
            if h == 1:
                r[0:128, :] = 0.0               # wrapped queries: garbage
            num += r[:, 0:H]
            den += r[:, H]
            ko = np.asarray(res.results[2 * b + h]["k_out"], np.float32)
            vo = np.asarray(res.results[2 * b + h]["v_out"], np.float32)
            i = np.arange(128)
            for p in range(8):
                K[b, 512 * p + 128 * h + i] = ko[0:H, 128 * p + i].T
                K[b, 512 * p + 256 + 128 * h + i] = ko[H:2 * H, 128 * p + i].T
            idx = (np.arange(NKC) * 256 + 128 * h)[:, None] + i
            V[b, idx.ravel()] = vo.reshape(S // 2, H)
        result[b] = (num / den[:, None]).astype(np.float32)
    K += bk
    V += bv
    result += bv
    return result, K, V
